# revision 46
# baseline (speedup 1.0000x reference)
"""Trainium2 Bass kernel for:
    logits4 = einsum('bic,bjc->bijc', Q, K) + bias      # [B,I,J,C]
    output  = sigmoid(logits4).mean(axis=-2)            # [B,I,C]
    attention_logits = einsum('bic,bjc->bij', Q, K)     # [B,I,J]
    return (output, attention_logits)

B,I,J,C = 4,512,512,512. Runs SPMD on 8 NeuronCores: core k handles
(b = k//2, h = k%2) with the sigmoid-mean part sharded over C-halves
(all I/J), and the attention-logits matmul sharded over I-halves (all C).

DESIGN="dve" (default) per-core dataflow, per c of the core's 256:
  - TensorE builds the biased outer product P[i, j] = Q[i,c]*K[j,c] +
    bias[c] as 4 contraction-dim-2 matmuls (one per 128-row i-block):
    lhsT = (Q^T row c | ones), rhs = (K^T row c | bias[c]*ones), into
    one [128, 2048] f32 PSUM group (4 banks, double buffered).
  - ScalarE: ONE sigmoid ACTIVATE per c over the whole group via a
    [2,1024] 3D AP (flat 2048 hangs the engine; [2,1024] is HW-legal).
    Measured on this backend an ACTIVATE costs ~0.9us nearly
    independent of N, so one-instruction-per-c is the dominant win
    (the 2549804ns baseline used N=512 -> 4 instructions per c).
  - VectorE reduces over j: pairwise 2x tensor_adds then one 1x
    tensor_reduce into a c-major f32 stage; the stage is DMA'd raw
    (contiguous) and the host reassembles [I, CH]. DVE post-op DRAIN
    makes any DVE op cost ~2x its streaming time, so this chain is
    ~2.0us/c and is the measured bottleneck; batching more c's per
    chain (BATCH_C) is throughput-neutral (drain scales with op size).
  - attention_logits: QK^T matmuls issued mid-loop (chunk LOGITS_AT),
    reusing an mp "ps" slot; DVE copies PSUM->SBUF; DMA out.
DESIGN="pe" reduces over j with (1/J)*ones matmuls in a transposed
[j, i] layout instead — measured 2x SLOWER here: the PSUM slot-release
chain (outer mms -> ACT -> reduce-mms -> DVE copy) gates the in-order
PE queue with only 2 slots available.
"""
import os

if "JAX_PLATFORMS" in os.environ and "axon" not in os.environ["JAX_PLATFORMS"]:
    # the bass kernel executes through the axon PJRT backend
    os.environ["JAX_PLATFORMS"] = ""

import numpy as np
import ml_dtypes

import concourse.bacc as bacc
import concourse.mybir as mybir
from concourse import tile
from concourse.bass_utils import run_bass_kernel_spmd

B, I, J, C = 4, 512, 512, 512
NCORES = 8
CH = C // 2          # c-half per core
IH = I // 2          # i-half per core
NIB = I // 128       # 128-blocks per i (or j) axis (4)
CHUNK = 16           # c's per staged operand tile (16 measures ~45us faster
                     # than 8: fewer chunk-boundary DMA/sem stalls)

BF16 = mybir.dt.bfloat16
F32 = mybir.dt.float32
ADD = mybir.AluOpType.add

DESIGN = "dve"        # "pe": TensorE ones-matmul j-reduction (no DVE in loop)
                      # "dve": VectorE add/add/reduce chain ([i,j] layout)
ACT_N = 2048          # free elems per ACTIVATE ([2,1024] AP is HW-legal; flat 2048 hangs)
GROUP_BANKS = 4       # PSUM banks per matmul/ACT group ("dve": 2 or 4)
DVE_ADDS = 4          # ("dve") pairwise TT halvings before the 1x tensor_reduce
BATCH_C = 1           # ("dve") c's per DVE reduce chain (1/4/8 all equal on HW)
LOGITS_AT = 1         # chunk index at which to issue the attention-logits work
DRAIN_AT = {9: (0, 128), 15: (128, 224)}  # ("dve") chunk -> mean cols to drain
DRAIN_TAIL = 224      # ("dve") columns drained after the loop
ST_BUFS = 4           # staging buffers for qk slabs (prefetch depth)
SPLIT_DMA = 0         # 1 = load each chunk slab as two DMAs (A/B only)
SG_BUFS = 3           # buffers for sigw / t reduce temporaries
PASSES = 1            # repeat the main loop (timing experiments only)


def build_nc():
    nc = bacc.Bacc("TRN2", target_bir_lowering=False, debug=False, num_devices=NCORES)

    # DESIGN="pe":  qp row0 = Q^T[c,:] (rhs), row1 = bias[c] broadcast
    #               kp row0 = K^T[c,:] (lhsT), row1 = 1.0
    # DESIGN="dve": qp row0 = Q^T[c,:] (lhsT), row1 = 1.0
    #               kp row0 = K^T[c,:] (rhs), row1 = bias[c] broadcast
    # qp/kp packed per chunk: slab k = [CHUNK*I q-operands | CHUNK*J k-ops]
    qk = nc.dram_tensor("qk", [2, CH * (I + J)], BF16, kind="ExternalInput")
    qt = nc.dram_tensor("qt", [C, IH], BF16, kind="ExternalInput")   # Q^T, i-half
    kt = nc.dram_tensor("kt", [C, J], BF16, kind="ExternalInput")    # K^T, full
    rone = nc.dram_tensor("rone", [128, 1], BF16, kind="ExternalInput")  # 1/J
    if DESIGN == "pe":
        out_mean = nc.dram_tensor("out_mean", [CH, I], F32, kind="ExternalOutput")
    else:
        # raw c-major stage dump [p, cc*NIB+ib]; host reassembles to [I, CH]
        out_mean = nc.dram_tensor(
            "out_mean", [128, CH * NIB], F32, kind="ExternalOutput"
        )
    out_logits = nc.dram_tensor("out_logits", [IH, J], F32, kind="ExternalOutput")

    GB = GROUP_BANKS if DESIGN == "dve" else 4
    GN = GB * 512            # free elems per PSUM group
    NG = NIB * J // GN       # groups per c
    MP_BUFS = 3 if GB == 2 else 2

    with tile.TileContext(nc) as tc:
        with (
            tc.tile_pool(name="sb", bufs=1) as sb,
            tc.tile_pool(name="st", bufs=3) as st,
            tc.tile_pool(name="mp", bufs=MP_BUFS, space="PSUM") as mp,
            tc.tile_pool(name="lp", bufs=1, space="PSUM") as lp,
            tc.tile_pool(name="sg", bufs=3) as sg,
        ):
            # main-loop chunk 0/1 operands first so PE can start immediately
            SL = CHUNK * (I + J)      # packed slab size
            KOFF = CHUNK * I          # k-operand offset inside a slab
            pre_qk = []
            for chunk in range(2):
                qkt = st.tile([2, SL], BF16, tag="qk", bufs=ST_BUFS, name="qkt")
                if SPLIT_DMA:
                    nc.sync.dma_start(
                        qkt[:, :KOFF], qk[:, chunk * SL : chunk * SL + KOFF]
                    )
                    nc.sync.dma_start(
                        qkt[:, KOFF:], qk[:, chunk * SL + KOFF : (chunk + 1) * SL]
                    )
                else:
                    nc.sync.dma_start(qkt[:], qk[:, chunk * SL : (chunk + 1) * SL])
                pre_qk.append(qkt)

            ones_r = sb.tile([128, 1], BF16, tag="ones_r")
            nc.sync.dma_start(ones_r[:], rone[:])

            qt_t = []
            kt_t = []
            for t in range(C // 128):
                a = sb.tile([128, IH], BF16, tag=f"qt{t}")
                nc.sync.dma_start(a[:], qt[128 * t : 128 * (t + 1), :])
                qt_t.append(a)
                b = sb.tile([128, J], BF16, tag=f"kt{t}")
                nc.sync.dma_start(b[:], kt[128 * t : 128 * (t + 1), :])
                kt_t.append(b)

            # "dve" design: means land here, c-major so the batched reduce
            # writes a contiguous slice: stage[p, cc*NIB+ib] = mean[ib*128+p, cc]
            stage = None
            if DESIGN == "dve":
                stage = sb.tile([128, CH * NIB], F32, tag="stage")

            pend = None          # (c, ps, sigT) awaiting its j-reduction
            stag_box = [None]    # current [1, CHUNK*I] SBUF staging row

            def emit_reduce(rc, ps, sigT):
                # mean over j: 4 accumulating ones-matmuls into the (already
                # consumed) group's own bank 0, row 0; bounce PSUM->SBUF via
                # one small DVE copy; DMA a CHUNK of mean rows at once.
                m = rc % CHUNK
                if m == 0:
                    stag_box[0] = st.tile(
                        [1, CHUNK * I], F32, tag="stag", name="stag"
                    )
                stag = stag_box[0]
                # acc lives in the group's LAST bank: the next-next c's outer
                # matmuls write banks 0-2 gated only on this c's ACT read,
                # and only its 4th matmul waits for the drain chain below
                acc = ps[0:1, 1536:2048]
                for jb in range(NIB):
                    nc.tensor.matmul(
                        acc,
                        ones_r[:],
                        sigT[:, jb * 512 : (jb + 1) * 512],
                        start=(jb == 0),
                        stop=(jb == NIB - 1),
                    )
                nc.vector.tensor_copy(stag[:, m * I : (m + 1) * I], acc)
                if m == CHUNK - 1:
                    nc.sync.dma_start(
                        out_mean[:].rearrange("a b -> () (a b)")[
                            :, (rc - m) * I : (rc + 1) * I
                        ],
                        stag[:],
                    )

            def do_logits():
                # GB=2: dedicated 2-bank tile; else reuse an mp "ps" slot
                # (same tag+shape so the pool doesn't grow past 8 banks).
                if GB == 2:
                    ps_lg = lp.tile([128, 2 * J], F32, tag="lg")
                else:
                    ps_lg = mp.tile([128, GN], F32, tag="ps")
                for it in range(IH // 128):
                    for cb in range(C // 128):
                        nc.tensor.matmul(
                            ps_lg[:, it * J : (it + 1) * J],
                            qt_t[cb][:, it * 128 : (it + 1) * 128],
                            kt_t[cb][:],
                            start=(cb == 0),
                            stop=(cb == C // 128 - 1),
                        )
                for it in range(IH // 128):
                    lg = sb.tile([128, J], F32, tag=f"lg{it}")
                    nc.vector.tensor_copy(lg[:], ps_lg[:, it * J : (it + 1) * J])
                    nc.sync.dma_start(out_logits[it * 128 : (it + 1) * 128, :], lg[:])

            for _ in range(PASSES):
              for chunk in range(CH // CHUNK):
                c0 = chunk * CHUNK
                if chunk < 2:
                    qkt = pre_qk[chunk]
                else:
                    qkt = st.tile([2, SL], BF16, tag="qk", bufs=ST_BUFS, name="qkt")
                    if SPLIT_DMA:
                        nc.sync.dma_start(
                            qkt[:, :KOFF], qk[:, chunk * SL : chunk * SL + KOFF]
                        )
                        nc.sync.dma_start(
                            qkt[:, KOFF:],
                            qk[:, chunk * SL + KOFF : (chunk + 1) * SL],
                        )
                    else:
                        nc.sync.dma_start(
                            qkt[:], qk[:, chunk * SL : (chunk + 1) * SL]
                        )
                qs, ks = qkt, qkt
                if chunk == LOGITS_AT:
                    do_logits()
                if DESIGN == "dve" and chunk in DRAIN_AT:
                    lo, hi = DRAIN_AT[chunk]
                    nc.vector.tensor_scalar_mul(
                        stage[:, lo * NIB : hi * NIB],
                        stage[:, lo * NIB : hi * NIB],
                        1.0 / J,
                    )
                    nc.sync.dma_start(
                        out_mean[:, lo * NIB : hi * NIB],
                        stage[:, lo * NIB : hi * NIB],
                    )
                for m in range(CHUNK):
                    c = c0 + m
                    if DESIGN == "pe":
                        # transposed outer product: ps[j, (jb, i)] for this c
                        ps = mp.tile([128, GN], F32, tag="ps")
                        for jb in range(NIB):
                            nc.tensor.matmul(
                                ps[:, jb * 512 : (jb + 1) * 512],
                                ks[
                                    :,
                                    KOFF + m * J + jb * 128 : KOFF
                                    + m * J
                                    + (jb + 1) * 128,
                                ],
                                qs[:, m * I : (m + 1) * I],
                                start=True,
                                stop=True,
                            )
                        sigT = sg.tile([128, GN], BF16, tag="sigT")
                        src = ps[:].rearrange("p (t n) -> p t n", t=GN // 1024)
                        dst = sigT[:].rearrange("p (t n) -> p t n", t=GN // 1024)
                        nc.scalar.activation(
                            dst, src, mybir.ActivationFunctionType.Sigmoid
                        )
                        # software-pipelined by one c: reduce the PREVIOUS c
                        # here so its ACT-dependent ones-matmuls don't block
                        # this c's outer matmuls in the in-order PE queue
                        if pend is not None:
                            emit_reduce(*pend)
                        pend = (c, ps, sigT)
                        continue
                    bm = c % BATCH_C
                    if bm == 0:
                        sigw = sg.tile(
                            [128, BATCH_C * NIB * J], BF16, tag="sigw",
                            name="sigw", bufs=SG_BUFS,
                        )
                    for g in range(NG):
                        ps = mp.tile([128, GN], F32, tag="ps")
                        for ib in range(GB):
                            nc.tensor.matmul(
                                ps[:, ib * J : (ib + 1) * J],
                                qs[
                                    :,
                                    m * I
                                    + (g * GB + ib) * 128 : m * I
                                    + (g * GB + ib + 1) * 128,
                                ],
                                ks[:, KOFF + m * J : KOFF + (m + 1) * J],
                                start=True,
                                stop=True,
                            )
                        for a0 in range(0, GN, ACT_N):
                            w0 = bm * NIB * J + g * GN + a0
                            src = ps[:, a0 : a0 + ACT_N]
                            dst = sigw[:, w0 : w0 + ACT_N]
                            if ACT_N > 1024:
                                tt = ACT_N // 1024
                                src = src.rearrange("p (t n) -> p t n", t=tt)
                                dst = dst.rearrange("p (t n) -> p t n", t=tt)
                            nc.scalar.activation(
                                dst, src, mybir.ActivationFunctionType.Sigmoid
                            )
                    if bm == BATCH_C - 1:
                        # DVE: one batched chain over BATCH_C c's — pairwise
                        # 2x adds, then a single 1x reduce into c-major stage
                        G = BATCH_C * NIB
                        cur = sigw[:].rearrange("p (g j) -> p g j", g=G)
                        w = J
                        for _a in range(DVE_ADDS):
                            t1 = sg.tile(
                                [128, G * (w // 2)], BF16, tag=f"t{_a}",
                                name=f"t{_a}", bufs=SG_BUFS,
                            )
                            t13 = t1[:].rearrange("p (g j) -> p g j", g=G)
                            nc.vector.tensor_add(
                                t13, cur[:, :, : w // 2], cur[:, :, w // 2 :]
                            )
                            cur = t13
                            w //= 2
                        nc.vector.tensor_reduce(
                            stage[:, (c - bm) * NIB : (c + 1) * NIB].rearrange(
                                "p g -> p g ()"
                            ),
                            cur,
                            axis=mybir.AxisListType.X,
                            op=ADD,
                        )

            if DESIGN == "pe":
                if pend is not None:
                    emit_reduce(*pend)
                    pend = None
            else:
                lo = DRAIN_TAIL
                nc.vector.tensor_scalar_mul(
                    stage[:, lo * NIB :], stage[:, lo * NIB :], 1.0 / J
                )
                nc.sync.dma_start(
                    out_mean[:, lo * NIB :], stage[:, lo * NIB :]
                )

    nc.compile()
    return nc


def make_in_maps(Q, K, bias):
    Q = np.asarray(Q, dtype=np.float32)
    K = np.asarray(K, dtype=np.float32)
    bias = np.asarray(bias, dtype=np.float32)
    qts = [np.ascontiguousarray(Q[b].T).astype(ml_dtypes.bfloat16) for b in range(B)]
    kts = [np.ascontiguousarray(K[b].T).astype(ml_dtypes.bfloat16) for b in range(B)]
    rone = np.full((128, 1), 1.0 / J, dtype=ml_dtypes.bfloat16)
    in_maps = []
    for core in range(NCORES):
        b, h = core // 2, core % 2
        cs = slice(h * CH, (h + 1) * CH)
        QT = qts[b]  # [C, I]
        KT = kts[b]  # [C, J]
        bias_h = bias[cs].astype(ml_dtypes.bfloat16)[:, None]
        qp = np.empty((2, CH, I), dtype=ml_dtypes.bfloat16)
        kp = np.empty((2, CH, J), dtype=ml_dtypes.bfloat16)
        qp[0] = QT[cs]
        kp[0] = KT[cs]
        if DESIGN == "pe":
            qp[1] = bias_h
            kp[1] = np.float32(1.0)
        else:
            qp[1] = np.float32(1.0)
            kp[1] = bias_h
        nch = CH // CHUNK
        qk = np.concatenate(
            [
                qp.reshape(2, nch, CHUNK * I),
                kp.reshape(2, nch, CHUNK * J),
            ],
            axis=2,
        )
        in_maps.append(
            {
                "qk": qk.reshape(2, CH * (I + J)),
                "qt": np.ascontiguousarray(QT[:, h * IH : (h + 1) * IH]),
                "kt": np.ascontiguousarray(KT),
                "rone": rone,
            }
        )
    return in_maps


def assemble(results):
    output = np.empty((B, I, C), dtype=np.float32)
    attention_logits = np.empty((B, I, J), dtype=np.float32)
    for core in range(NCORES):
        b, h = core // 2, core % 2
        om = results[core]["out_mean"]
        if DESIGN == "pe":
            om = om.T  # [CH, I] -> [I, CH]
        else:
            # raw [p, cc*NIB+ib] -> [I, CH]: i = ib*128 + p
            om = (
                om.reshape(128, CH, NIB)
                .transpose(2, 0, 1)
                .reshape(I, CH)
            )
        output[b, :, h * CH : (h + 1) * CH] = om
        attention_logits[b, h * IH : (h + 1) * IH, :] = results[core]["out_logits"]
    return output, attention_logits


def build_null_nc():
    """Minimal kernel used by test.py to measure dispatch overhead."""
    nc = bacc.Bacc("TRN2", target_bir_lowering=False, debug=False, num_devices=NCORES)
    x = nc.dram_tensor("x", [8, 8], F32, kind="ExternalInput")
    y = nc.dram_tensor("y", [8, 8], F32, kind="ExternalOutput")
    with tile.TileContext(nc) as tc:
        with tc.tile_pool(name="p", bufs=1) as pool:
            t = pool.tile([8, 8], F32)
            nc.sync.dma_start(t[:], x[:])
            nc.sync.dma_start(y[:], t[:])
    nc.compile()
    return nc


_NC = None


def get_nc():
    global _NC
    if _NC is None:
        _NC = build_nc()
    return _NC


def run(Q, K, bias, **kwargs):
    nc = get_nc()
    res = run_bass_kernel_spmd(
        nc, make_in_maps(Q, K, bias), core_ids=list(range(NCORES)), **kwargs
    )
    return res


def kernel(Q, K, bias):
    res = run(Q, K, bias)
    return assemble(res.results)
